# revision 1
# baseline (speedup 1.0000x reference)
"""3-layer GAT classifier on 8 TRN2 NeuronCores (Bass/Tile).

Strategy (per spec sharding hint): destination nodes are partitioned across
the 8 cores (2500 real nodes each, padded to 2560 = 20 tiles of 128).
Within a core, destinations are sorted by in-degree so each 128-dst tile has
near-uniform degree (padded-CSR with per-tile max degree D_t, globally
uniform across cores so the SPMD program is identical everywhere).

Per layer l:
  1. Each core computes h' = x @ [W | W@A_s | W@A_d] for its own shard on
     the PE (bf16; the A-extension columns give per-node attention scores
     a_s, a_d directly; W@A_* is computed on-device once per layer).
     It writes bf16 table rows [h + bias | a_s | pad] to a DRAM shard
     (bias folds into the table because softmax weights sum to 1).
     Row 2559 of every shard is a dummy row (h=0, a_s=-1e30) used for CSR
     padding slots.
  2. AllGather -> full gather table [20480, elem] on every core.
  3. Per dst tile: chunked dma_gather pulls all incident-edge rows
     (slot-major padded CSR, 128 edges per slot; <=1024 idx / ~1MB per op,
     a HW limit).  Segment softmax runs as ~8 fused broadcast-AP ops per
     tile (per-head structure handled by strided/step-0 APs; 1/den folds
     into alpha).  The weighted sum is a broadcast multiply (bf16) +
     strided reduce (fp32) per 16-slot chunk.  Output feeds the next
     layer's matmul via PE transpose, all on-chip.
Final: mean-pool by graph via one-hot matmul (PSUM-accumulated across
tiles), AllReduce, tiny FC -> [64, 3].

Host-side work is limited to index manipulation (edge grouping, padded CSR
construction, one-hot graph membership, 1/count) plus dtype/layout staging
of the inputs.
"""

import sys

sys.path.insert(0, "/opt/trn_rl_repo")

import numpy as np
import ml_dtypes

N_NODES = 20000
N_EDGES = 320000
N_GRAPHS = 64
NC_ = 8          # cores
P = 128          # partitions
NPC = 2500       # real nodes per core
NSH = 2560       # padded shard rows per core
NT = NSH // P    # 20 dst tiles per core
V = NC_ * NSH    # 20480 gather-table rows
CHT = NT // 2    # tiles per AllGather chunk (2 chunks per layer)
CHR = CHT * P    # shard rows per AllGather chunk
DUMMY = NSH - 1  # shard row used for padding slots (core 0's)
NEG = -1.0e30

# (Fin, H, Fout, ELEM) per layer; ELEM = bf16 elements per table row,
# padded so ELEM*2 bytes is a multiple of 256.
LAYERS = [
    (384, 4, 512, 640),
    (512, 2, 256, 384),
    (256, 1, 128, 256),
]

SCHUNK = 16  # slots per weighted-sum chunk (bounds the `scaled` tile)
# slots (128 idxs each) per dma_gather op; HW crashes above ~1024 idxs or
# ~1.1MB per gather op (empirical, see probe_hw.py)
GCHUNK = [7, 8, 8]

BF16 = ml_dtypes.bfloat16

_CACHE = {}


# ----------------------------------------------------------------------------
# Host-side preprocessing (index manipulation only)
# ----------------------------------------------------------------------------

def _prep(x, edge_index, batch):
    x = np.asarray(x, dtype=np.float32)
    ei = np.asarray(edge_index)
    b_all = np.asarray(batch).astype(np.int64)

    loop = np.arange(N_NODES, dtype=np.int64)
    src = np.concatenate([ei[0].astype(np.int64), loop])
    dst = np.concatenate([ei[1].astype(np.int64), loop])

    cd = dst // NPC
    ld = dst % NPC

    deg = np.zeros((NC_, NPC), np.int64)
    np.add.at(deg, (cd, ld), 1)
    order = np.argsort(-deg, axis=1, kind="stable")      # [NC_, NPC]
    rank = np.empty_like(order)
    for c in range(NC_):
        rank[c, order[c]] = np.arange(NPC)

    degsort = np.take_along_axis(deg, order, axis=1)
    degsort = np.concatenate(
        [degsort, np.zeros((NC_, NSH - NPC), np.int64)], axis=1)
    D = [int(max(1, degsort[:, t * P:(t + 1) * P].max())) for t in range(NT)]
    ss = np.concatenate([[0], np.cumsum(D)]).astype(np.int64)
    S = int(ss[-1])

    nodes = np.arange(N_NODES)
    pos = (nodes // NPC) * NSH + rank[nodes // NPC, nodes % NPC]  # [N]

    # place each edge at (core, slot, partition); self-loop edge first
    # (slot 0 is loaded from the local shard by a plain DMA, not gathered)
    key = cd * NSH + rank[cd, ld]
    eo = np.lexsort(((src != dst).astype(np.int8), key))
    ks = key[eo]
    first = np.searchsorted(ks, ks, side="left")
    sidx = np.arange(len(ks)) - first                    # within-dst slot
    ce = ks // NSH
    re = ks % NSH
    te = re // P
    pe = re % P
    slot = ss[te] + sidx
    full = np.full((NC_, S, P), DUMMY, np.int16)
    full[ce, slot, pe] = pos[src[eo]].astype(np.int16)

    # dma_gather index layout: idx i at [i % 16, i // 16], replicated x8
    idxw = full.reshape(NC_, S * 8, 16).transpose(0, 2, 1)   # [NC_,16,S*8]
    idx16 = np.ascontiguousarray(np.tile(idxw, (1, 8, 1)))   # [NC_,128,S*8]

    # x in pos order, transposed for the L1 matmul: [c, 128f, 3kc, NSH]
    xp = np.zeros((NC_, NSH, 384), np.float32)
    for c in range(NC_):
        xp[c, rank[c], :] = x[c * NPC:(c + 1) * NPC]
    xT = np.ascontiguousarray(
        xp.transpose(0, 2, 1).reshape(NC_, 3, P, NSH).transpose(0, 2, 1, 3)
    ).astype(BF16)

    # graph one-hot per core (zero rows for padding nodes) + 1/count
    oh = np.zeros((NC_, NSH, N_GRAPHS), np.float32)
    for c in range(NC_):
        oh[c][rank[c], b_all[c * NPC:(c + 1) * NPC]] = 1.0
    oh = oh.astype(BF16)
    cnt = np.bincount(b_all, minlength=N_GRAPHS).astype(np.float32)
    inv_cnt = (1.0 / np.maximum(cnt, 1.0)).reshape(N_GRAPHS, 1)

    return dict(D=D, ss=ss, S=S, idx16=idx16, xT=xT, oh=oh, inv_cnt=inv_cnt)


# ----------------------------------------------------------------------------
# Device program
# ----------------------------------------------------------------------------

def _build(D, S, reps=1, skip_ag=False, skip_gather=False, skip_mac=False):
    import concourse.bass as bass
    import concourse.mybir as mybir
    import concourse.tile as tile
    from concourse import bacc
    from concourse.masks import make_identity

    f32 = mybir.dt.float32
    bf16 = mybir.dt.bfloat16
    i16 = mybir.dt.int16
    Alu = mybir.AluOpType
    Act = mybir.ActivationFunctionType
    ss = np.concatenate([[0], np.cumsum(D)]).astype(int)

    nc = bacc.Bacc("TRN2", target_bir_lowering=False, debug=False,
                   num_devices=NC_)

    # ---- I/O ----
    xT = nc.dram_tensor("xT", [P, 3, NSH], bf16, kind="ExternalInput")
    idx = nc.dram_tensor("idx", [P, S * 8], i16, kind="ExternalInput")
    Ws, atts, brs = [], [], []
    for li, (Fin, H, Fout, ELEM) in enumerate(LAYERS):
        Ws.append(nc.dram_tensor(f"W{li}", [Fin, Fout], f32,
                                 kind="ExternalInput"))
        atts.append(nc.dram_tensor(f"att{li}", [Fout, 2 * H], f32,
                                   kind="ExternalInput"))
        brs.append(nc.dram_tensor(f"b{li}", [P, Fout], f32,
                                  kind="ExternalInput"))
    oh = nc.dram_tensor("oh", [NSH, N_GRAPHS], bf16, kind="ExternalInput")
    fcW = nc.dram_tensor("fcW", [P, 3], f32, kind="ExternalInput")
    fcb = nc.dram_tensor("fcb", [N_GRAPHS, 3], f32, kind="ExternalInput")
    invc = nc.dram_tensor("invc", [N_GRAPHS, 1], f32, kind="ExternalInput")
    out = nc.dram_tensor("out", [N_GRAPHS, 3], f32, kind="ExternalOutput")

    rg = [list(range(NC_))]

    with tile.TileContext(nc) as tc:
        with tc.tile_pool(name="const", bufs=1) as cpool, \
             tc.tile_pool(name="dram", bufs=1, space="DRAM") as dram, \
             tc.tile_pool(name="io", bufs=3) as io, \
             tc.tile_pool(name="gth", bufs=2) as gth, \
             tc.tile_pool(name="cmp", bufs=2) as cmp, \
             tc.tile_pool(name="ps", bufs=2, space="PSUM") as ps, \
             tc.tile_pool(name="pacc", bufs=1, space="PSUM") as pacc:

            # ---- DRAM internals ----
            shard = [dram.tile([NSH, ELEM], bf16, name=f"shard{li}")
                     for li, (_, _, _, ELEM) in enumerate(LAYERS)]
            pool_in = dram.tile([N_GRAPHS, P], f32, name="pool_in")

            # ---- constants to SBUF ----
            idx_sb = cpool.tile([P, S * 8], i16, name="idx_sb")
            nc.sync.dma_start(idx_sb[:], idx[:])

            b_sb, ad_all, dummy = [], [], []
            for li, (Fin, H, Fout, ELEM) in enumerate(LAYERS):
                t_ = cpool.tile([P, Fout], f32, name=f"b_sb{li}")
                nc.sync.dma_start(t_[:], brs[li][:])
                b_sb.append(t_)
                ad_all.append(cpool.tile([P, NT * H], f32, name=f"ad{li}"))
                dm = cpool.tile([1, ELEM], bf16, name=f"dummy{li}")
                nc.vector.memset(dm[:], 0.0)
                nc.vector.memset(dm[:, Fout:Fout + H], NEG)
                dummy.append(dm)

            ident_bf = cpool.tile([P, P], bf16, name="ident_bf")
            make_identity(nc, ident_bf[:])
            ident_f = cpool.tile([P, P], f32, name="ident_f")
            make_identity(nc, ident_f[:])
            fcW_sb = cpool.tile([P, 3], f32, name="fcW_sb")
            nc.sync.dma_start(fcW_sb[:], fcW[:])
            fcb_sb = cpool.tile([N_GRAPHS, 3], f32, name="fcb_sb")
            nc.sync.dma_start(fcb_sb[:], fcb[:])
            invc_sb = cpool.tile([N_GRAPHS, 1], f32, name="invc_sb")
            nc.sync.dma_start(invc_sb[:], invc[:])

            # persistent PSUM accumulator for graph pooling
            pool_ps = pacc.tile([N_GRAPHS, P], f32, name="pool_ps")

            # ---- W_ext = [W | W@A_s | W@A_d] (bf16, per layer) ----
            def build_wext(li, rep):
                Fin, H, Fout, ELEM = LAYERS[li]
                KC, FoC = Fin // P, Fout // P
                wext = cpool.tile([P, KC, Fout + 2 * H], bf16,
                                  tag=f"wext{li}", name=f"wext{li}_r{rep}")
                nc.gpsimd.dma_start(
                    wext[:, :, :Fout],
                    Ws[li].ap().rearrange("(k p) f -> p k f", p=P))
                att_sb = cpool.tile([P, FoC, 2 * H], bf16,
                                    tag=f"attsb{li}", name=f"attsb{li}_r{rep}")
                nc.gpsimd.dma_start(
                    att_sb[:],
                    atts[li].ap().rearrange("(c p) h -> p c h", p=P))
                for fic in range(KC):
                    wa_ps = ps.tile([P, 2 * H], f32, tag="hpa")
                    for foc in range(FoC):
                        tp = ps.tile([P, P], bf16, tag="tp")
                        nc.tensor.transpose(
                            tp[:], wext[:, fic, foc * P:(foc + 1) * P],
                            ident_bf[:])
                        wt = cmp.tile([P, P], bf16, tag="wt")
                        nc.vector.tensor_copy(wt[:], tp[:])
                        nc.tensor.matmul(
                            wa_ps[:], lhsT=wt[:], rhs=att_sb[:, foc, :],
                            start=(foc == 0), stop=(foc == FoC - 1))
                    nc.vector.tensor_copy(wext[:, fic, Fout:], wa_ps[:])
                return wext

            def do_ag(li, tbl):
                if skip_ag:
                    nc.sync.dma_start(tbl[li][:NSH], shard[li][:])
                    return
                nc.gpsimd.collective_compute(
                    "AllGather", Alu.bypass, replica_groups=rg,
                    ins=[shard[li][:]], outs=[tbl[li][:]])

            # ---- shared tail: h/a in PSUM -> table row + local a_d ----
            def layer_tail(li, t, hp, hpa):
                Fin, H, Fout, ELEM = LAYERS[li]
                nc.scalar.copy(
                    ad_all[li][:, t * H:(t + 1) * H], hpa[:, H:2 * H])
                row = io.tile([P, ELEM], bf16, tag="row")
                nc.vector.tensor_tensor(
                    out=row[:, :Fout], in0=hp[:, :Fout], in1=b_sb[li][:],
                    op=Alu.add)
                nc.scalar.copy(row[:, Fout:Fout + H], hpa[:, :H])
                if ELEM > Fout + H:
                    nc.vector.memset(row[:, Fout + H:], 0.0)
                nc.sync.dma_start(shard[li][t * P:(t + 1) * P, :], row[:])

            for rep in range(reps):
                tbl = [dram.tile([V, ELEM], bf16, addr_space="Shared",
                                 name=f"tbl{li}_r{rep}")
                       for li, (_, _, _, ELEM) in enumerate(LAYERS)]
                pool_out = dram.tile([N_GRAPHS, P], f32, addr_space="Shared",
                                     name=f"pool_out_r{rep}")
                wexts = [build_wext(li, rep) for li in range(len(LAYERS))]

                # ---- stage 0: L1 matmul over own shard ----
                for t in range(NT):
                    xt = io.tile([P, 3, P], bf16, tag="xt")
                    nc.sync.dma_start(xt[:], xT[:, :, t * P:(t + 1) * P])
                    Fout0, H0 = LAYERS[0][2], LAYERS[0][1]
                    hp = ps.tile([P, Fout0], f32, tag="hp")
                    hpa = ps.tile([P, 2 * H0], f32, tag="hpa")
                    for kc in range(3):
                        nc.tensor.matmul(
                            hp[:], lhsT=xt[:, kc, :],
                            rhs=wexts[0][:, kc, :Fout0],
                            start=(kc == 0), stop=(kc == 2))
                        nc.tensor.matmul(
                            hpa[:], lhsT=xt[:, kc, :],
                            rhs=wexts[0][:, kc, Fout0:],
                            start=(kc == 0), stop=(kc == 2))
                    layer_tail(0, t, hp, hpa)
                nc.sync.dma_start(shard[0][DUMMY:DUMMY + 1, :], dummy[0][:])
                do_ag(0, tbl)

                # ---- stages 1..3: aggregate layer li, then matmul li+1 ----
                for li, (Fin, H, Fout, ELEM) in enumerate(LAYERS):
                    last = li == len(LAYERS) - 1
                    for t in range(NT):
                        Dt = D[t]
                        g = gth.tile([P, Dt, ELEM], bf16, tag="g")
                        nc.sync.dma_start(
                            g[:, 0, :], shard[li][t * P:(t + 1) * P, :])
                        if skip_gather:
                            pass
                        else:
                            for gs in range(1, Dt, GCHUNK[li]):
                                gch = min(GCHUNK[li], Dt - gs)
                                nc.gpsimd.dma_gather(
                                    g[:, gs:gs + gch, :], tbl[li][:],
                                    idx_sb[:, int(ss[t] + gs) * 8:
                                           int(ss[t] + gs + gch) * 8],
                                    num_idxs=P * gch, num_idxs_reg=P * gch,
                                    elem_size=ELEM, elem_step=ELEM)

                        # fused segment softmax; e layout [P, Dt, H]
                        e0 = cmp.tile([P, Dt, H], f32, tag="e0")
                        ad_b = ad_all[li][:, t * H:(t + 1) * H] \
                            .unsqueeze(1).broadcast_to([P, Dt, H])
                        nc.vector.tensor_tensor(
                            out=e0[:], in0=g[:, :, Fout:Fout + H],
                            in1=ad_b, op=Alu.add)
                        e1 = cmp.tile([P, Dt, H], f32, tag="e1")
                        nc.vector.scalar_tensor_tensor(
                            out=e1[:], in0=e0[:], scalar=0.2, in1=e0[:],
                            op0=Alu.mult, op1=Alu.max)
                        negm = cmp.tile([P, H], f32, tag="negm")
                        nc.vector.tensor_reduce(
                            out=negm[:],
                            in_=e1[:].rearrange("p s h -> p h s"),
                            axis=mybir.AxisListType.X, op=Alu.max,
                            negate=True)
                        negm_b = negm[:].unsqueeze(1).broadcast_to(
                            [P, Dt, H])
                        nc.vector.tensor_tensor(
                            out=e0[:], in0=e1[:], in1=negm_b, op=Alu.add)
                        w_ = cmp.tile([P, Dt, H], f32, tag="w_")
                        nc.scalar.activation(w_[:], e0[:], Act.Exp)
                        den = cmp.tile([P, H], f32, tag="den")
                        nc.vector.tensor_reduce(
                            out=den[:],
                            in_=w_[:].rearrange("p s h -> p h s"),
                            axis=mybir.AxisListType.X, op=Alu.add)
                        invden = cmp.tile([P, H], f32, tag="invden")
                        nc.vector.reciprocal(invden[:], den[:])
                        # alpha duplicated pairwise so the MAC multiply's
                        # operands are all innermost-packed (DVE 2x mode)
                        alpha = cmp.tile([P, Dt, H, 2], bf16, tag="alpha")
                        w_b = w_[:].unsqueeze(3).broadcast_to([P, Dt, H, 2])
                        invden_b = invden[:].unsqueeze(1).broadcast_to(
                            [P, Dt, H]).unsqueeze(3).broadcast_to(
                            [P, Dt, H, 2])
                        nc.vector.tensor_tensor(
                            out=alpha[:], in0=w_b, in1=invden_b,
                            op=Alu.mult)

                        # acc[p,f] = sum_s alpha[p,s,head(f)] * h[p,s,f]
                        acc = cmp.tile([P, Fout], f32, tag="acc")
                        acc2 = cmp.tile([P, Fout], f32, tag="acc2")
                        if skip_mac:
                            nc.vector.tensor_copy(acc[:], g[:, 0, :Fout])
                        for ci, cs in enumerate(
                                [] if skip_mac else
                                list(range(0, Dt, SCHUNK))):
                            ch = min(SCHUNK, Dt - cs)
                            sc = cmp.tile([P, SCHUNK * Fout], bf16, tag="sc")
                            g_v = g[:, cs:cs + ch, :Fout].rearrange(
                                "p c (h j k) -> p c h j k", h=H, j=P // 2)
                            w_v = alpha[:, cs:cs + ch, :, :].unsqueeze(
                                3).broadcast_to([P, ch, H, P // 2, 2])
                            s_v = sc[:, :ch * Fout].rearrange(
                                "p (c h j k) -> p c h j k",
                                c=ch, h=H, j=P // 2)
                            nc.vector.tensor_tensor(
                                out=s_v, in0=g_v, in1=w_v, op=Alu.mult)
                            # slot-sum as a tree of packed bf16 adds
                            # (tensor_reduce has no fast DVE mode)
                            cur = ch
                            while cur > 1:
                                hh = cur // 2
                                lo = cur - hh
                                nc.vector.tensor_tensor(
                                    out=sc[:, :hh * Fout],
                                    in0=sc[:, :hh * Fout],
                                    in1=sc[:, lo * Fout:(lo + hh) * Fout],
                                    op=Alu.add)
                                cur = lo
                            if ci == 0:
                                nc.scalar.copy(acc[:], sc[:, :Fout])
                            else:
                                nc.vector.tensor_tensor(
                                    out=acc[:], in0=acc[:], in1=sc[:, :Fout],
                                    op=Alu.add)

                        # x_next = elu(acc)   (bias already in table rows)
                        txm = cmp.tile([P, Fout], f32, tag="txm")
                        nc.vector.tensor_scalar_min(txm[:], acc[:], 0.0)
                        texp = cmp.tile([P, Fout], f32, tag="texp")
                        nc.scalar.activation(texp[:], txm[:], Act.Exp)
                        txr = cmp.tile([P, Fout], f32, tag="txr")
                        nc.scalar.activation(txr[:], acc[:], Act.Relu)
                        xn = cmp.tile([P, Fout], bf16, tag="xn")
                        nc.vector.scalar_tensor_tensor(
                            out=xn[:], in0=texp[:], scalar=-1.0, in1=txr[:],
                            op0=Alu.add, op1=Alu.add)

                        if not last:
                            Fin2, H2, Fout2, ELEM2 = LAYERS[li + 1]
                            KC2 = Fin2 // P
                            xtT = io.tile([P, KC2, P], bf16, tag="xtT")
                            for kc in range(KC2):
                                tp = ps.tile([P, P], bf16, tag="tp")
                                nc.tensor.transpose(
                                    tp[:], xn[:, kc * P:(kc + 1) * P],
                                    ident_bf[:])
                                nc.scalar.copy(xtT[:, kc, :], tp[:])
                            hp2 = ps.tile([P, Fout2], f32, tag="hp")
                            hpa2 = ps.tile([P, 2 * H2], f32, tag="hpa")
                            for kc in range(KC2):
                                nc.tensor.matmul(
                                    hp2[:], lhsT=xtT[:, kc, :],
                                    rhs=wexts[li + 1][:, kc, :Fout2],
                                    start=(kc == 0), stop=(kc == KC2 - 1))
                                nc.tensor.matmul(
                                    hpa2[:], lhsT=xtT[:, kc, :],
                                    rhs=wexts[li + 1][:, kc, Fout2:],
                                    start=(kc == 0), stop=(kc == KC2 - 1))
                            layer_tail(li + 1, t, hp2, hpa2)
                        else:
                            oh_t = io.tile([P, N_GRAPHS], bf16, tag="oh_t")
                            nc.sync.dma_start(
                                oh_t[:], oh[t * P:(t + 1) * P, :])
                            nc.tensor.matmul(
                                pool_ps[:], lhsT=oh_t[:], rhs=xn[:],
                                start=(t == 0), stop=(t == NT - 1))

                    if not last:
                        nc.sync.dma_start(
                            shard[li + 1][DUMMY:DUMMY + 1, :],
                            dummy[li + 1][:])
                        do_ag(li + 1, tbl)

                # ---- finale: mean pool + FC ----
                pool_sb = cmp.tile([N_GRAPHS, P], f32, tag="pool_sb")
                nc.vector.tensor_copy(pool_sb[:], pool_ps[:])
                nc.sync.dma_start(pool_in[:], pool_sb[:])
                if skip_ag:
                    nc.sync.dma_start(pool_out[:], pool_in[:])
                else:
                    nc.gpsimd.collective_compute(
                        "AllReduce", Alu.add, replica_groups=rg,
                        ins=[pool_in[:]], outs=[pool_out[:]])
                pr = cmp.tile([N_GRAPHS, P], f32, tag="pr")
                nc.sync.dma_start(pr[:], pool_out[:])
                pm = cmp.tile([N_GRAPHS, P], f32, tag="pm")
                nc.vector.tensor_scalar_mul(pm[:], pr[:], invc_sb[:, 0:1])
                tp2 = ps.tile([P, N_GRAPHS], f32, tag="tp")
                nc.tensor.transpose(
                    tp2[:], pm[:], ident_f[:N_GRAPHS, :N_GRAPHS])
                pmT = cmp.tile([P, N_GRAPHS], f32, tag="pmT")
                nc.vector.tensor_copy(pmT[:], tp2[:])
                fc_ps = ps.tile([N_GRAPHS, 3], f32, tag="tp")
                nc.tensor.matmul(fc_ps[:], lhsT=pmT[:], rhs=fcW_sb[:],
                                 start=True, stop=True)
                res = cmp.tile([N_GRAPHS, 3], f32, tag="res")
                nc.vector.tensor_tensor(
                    out=res[:], in0=fc_ps[:], in1=fcb_sb[:], op=Alu.add)
                nc.sync.dma_start(out[:], res[:])

    nc.compile()
    return nc


# ----------------------------------------------------------------------------
# Entry point
# ----------------------------------------------------------------------------

def _make_in_maps(prep, params, fcW, fcb):
    common = {}
    for li, (W, as_, ad_, b_) in enumerate(params):
        Fin, H, Fout, ELEM = LAYERS[li]
        common[f"W{li}"] = np.ascontiguousarray(np.asarray(W, np.float32))
        # block-diagonal att matrices: att[k*128+j, k] = a_s[k, j] etc.
        att = np.zeros((Fout, 2 * H), np.float32)
        as_ = np.asarray(as_, np.float32)
        ad_ = np.asarray(ad_, np.float32)
        for k in range(H):
            att[k * P:(k + 1) * P, k] = as_[k]
            att[k * P:(k + 1) * P, H + k] = ad_[k]
        common[f"att{li}"] = att
        common[f"b{li}"] = np.tile(
            np.asarray(b_, np.float32).reshape(1, Fout), (P, 1))
    common["fcW"] = np.ascontiguousarray(np.asarray(fcW, np.float32))
    common["fcb"] = np.tile(
        np.asarray(fcb, np.float32).reshape(1, 3), (N_GRAPHS, 1))
    common["invc"] = prep["inv_cnt"].astype(np.float32)

    in_maps = []
    for c in range(NC_):
        m = dict(common)
        m["xT"] = np.ascontiguousarray(prep["xT"][c])
        m["idx"] = np.ascontiguousarray(prep["idx16"][c])
        m["oh"] = np.ascontiguousarray(prep["oh"][c])
        in_maps.append(m)
    return in_maps


def kernel(x, edge_index, batch, W1, as1, ad1, b1, W2, as2, ad2, b2,
           W3, as3, ad3, b3, fcW, fcb, _trace=False):
    from concourse.bass_utils import run_bass_kernel_spmd

    prep = _prep(x, edge_index, batch)
    D, S = prep["D"], prep["S"]

    key = (tuple(D), S)
    if key not in _CACHE:
        _CACHE[key] = _build(D, S)
    nc = _CACHE[key]

    params = [(W1, as1, ad1, b1), (W2, as2, ad2, b2), (W3, as3, ad3, b3)]
    in_maps = _make_in_maps(prep, params, fcW, fcb)

    res = run_bass_kernel_spmd(nc, in_maps, core_ids=list(range(NC_)),
                               trace=_trace)
    out = res.results[0]["out"].astype(np.float32)
    if _trace:
        kernel._last_results = res
    return out



# revision 20
# speedup vs baseline: 2.5507x; 2.5507x over previous
"""3-layer GAT classifier on 8 TRN2 NeuronCores (Bass/Tile).

Strategy (per spec sharding hint): destination nodes are partitioned across
the 8 cores (2500 real nodes each, padded to 2560 = 20 tiles of 128).
Within a core, destinations are sorted by in-degree so each 128-dst tile has
near-uniform degree (padded-CSR with per-tile max degree D_t, globally
uniform across cores so the SPMD program is identical everywhere).

Per layer l:
  1. Each core computes h' = x @ [W | W@A_s | W@A_d] for its own shard on
     the PE (bf16; the A-extension columns give per-node attention scores
     a_s, a_d directly; W@A_* is computed on-device once per layer).
     It writes bf16 table rows [h + bias | a_s | pad] to a DRAM shard
     (bias folds into the table because softmax weights sum to 1).
     Row 2559 of every shard is a dummy row (h=0, a_s=-1e30) used for CSR
     padding slots.
  2. AllGather -> full gather table [20480, elem] on every core.
  3. Per dst tile: chunked dma_gather pulls all incident-edge rows
     (slot-major padded CSR, 128 edges per slot; <=1024 idx / ~1MB per op,
     a HW limit).  Segment softmax runs as ~8 fused broadcast-AP ops per
     tile (per-head structure handled by strided/step-0 APs; 1/den folds
     into alpha).  The weighted sum is a broadcast multiply (bf16) +
     strided reduce (fp32) per 16-slot chunk.  Output feeds the next
     layer's matmul via PE transpose, all on-chip.
Final: mean-pool by graph via one-hot matmul (PSUM-accumulated across
tiles), AllReduce, tiny FC -> [64, 3].

Host-side work is limited to index manipulation (edge grouping, padded CSR
construction, one-hot graph membership, 1/count) plus dtype/layout staging
of the inputs.
"""

import sys

sys.path.insert(0, "/opt/trn_rl_repo")

import numpy as np
import ml_dtypes

N_NODES = 20000
N_EDGES = 320000
N_GRAPHS = 64
NC_ = 8          # cores
P = 128          # partitions
NPC = 2500       # real nodes per core
NSH = 2560       # padded shard rows per core
NT = NSH // P    # 20 dst tiles per core
V = NC_ * NSH    # 20480 gather-table rows
G = 4            # tiles per AllGather chunk (pipelined with compute)
GR = G * P       # shard rows per AllGather chunk
NCH = NT // G    # chunks per layer
DUMMY = NSH - 1  # shard row used for padding slots (core 0's)
# gather-table position of core 0's dummy shard row in the chunked layout
DUMMY_POS = ((NSH - 1) // GR) * NC_ * GR + (NSH - 1) % GR
NEG = -1.0e30

# (Fin, H, Fout, ELEM) per layer; ELEM = bf16 elements per table row,
# padded so ELEM*2 bytes is a multiple of 256.
LAYERS = [
    (384, 4, 512, 640),
    (512, 2, 256, 384),
    (256, 1, 128, 256),
]

SCHUNK = 16  # slots per weighted-sum chunk (bounds the `scaled` tile)
# slots (128 idxs each) per dma_gather op; HW crashes above ~1024 idxs or
# ~1.1MB per gather op (empirical, see probe_hw.py)
GCHUNK = [7, 8, 8]

BF16 = ml_dtypes.bfloat16

_CACHE = {}


# ----------------------------------------------------------------------------
# Host-side preprocessing (index manipulation only)
# ----------------------------------------------------------------------------

def _prep(x, edge_index, batch):
    x = np.asarray(x, dtype=np.float32)
    ei = np.asarray(edge_index)
    b_all = np.asarray(batch).astype(np.int64)

    loop = np.arange(N_NODES, dtype=np.int64)
    src = np.concatenate([ei[0].astype(np.int64), loop])
    dst = np.concatenate([ei[1].astype(np.int64), loop])

    cd = dst // NPC
    ld = dst % NPC

    deg = np.zeros((NC_, NPC), np.int64)
    np.add.at(deg, (cd, ld), 1)
    order = np.argsort(-deg, axis=1, kind="stable")      # [NC_, NPC]
    rank = np.empty_like(order)
    for c in range(NC_):
        rank[c, order[c]] = np.arange(NPC)

    degsort = np.take_along_axis(deg, order, axis=1)
    degsort = np.concatenate(
        [degsort, np.zeros((NC_, NSH - NPC), np.int64)], axis=1)
    D = [int(max(1, degsort[:, t * P:(t + 1) * P].max())) for t in range(NT)]
    ss = np.concatenate([[0], np.cumsum(D)]).astype(np.int64)
    S = int(ss[-1])

    nodes = np.arange(N_NODES)
    c_n = nodes // NPC
    r_n = rank[c_n, nodes % NPC]
    # chunked-AG table layout: [chunk][core][row-in-chunk]
    pos = (r_n // GR) * (NC_ * GR) + c_n * GR + (r_n % GR)  # [N]

    # place each edge at (core, slot, partition); self-loop edge first
    # (slot 0 is loaded from the local shard by a plain DMA, not gathered)
    key = cd * NSH + rank[cd, ld]
    eo = np.lexsort(((src != dst).astype(np.int8), key))
    ks = key[eo]
    first = np.searchsorted(ks, ks, side="left")
    sidx = np.arange(len(ks)) - first                    # within-dst slot
    ce = ks // NSH
    re = ks % NSH
    te = re // P
    pe = re % P
    slot = ss[te] + sidx
    full = np.full((NC_, S, P), DUMMY_POS, np.int16)
    full[ce, slot, pe] = pos[src[eo]].astype(np.int16)

    # dma_gather index layout: idx i at [i % 16, i // 16], replicated x8
    idxw = full.reshape(NC_, S * 8, 16).transpose(0, 2, 1)   # [NC_,16,S*8]
    idx16 = np.ascontiguousarray(np.tile(idxw, (1, 8, 1)))   # [NC_,128,S*8]

    # x in pos order, transposed for the L1 matmul: [c, 128f, 3kc, NSH]
    xp = np.zeros((NC_, NSH, 384), np.float32)
    for c in range(NC_):
        xp[c, rank[c], :] = x[c * NPC:(c + 1) * NPC]
    xT = np.ascontiguousarray(
        xp.transpose(0, 2, 1).reshape(NC_, 3, P, NSH).transpose(0, 2, 1, 3)
    ).astype(BF16)

    # graph one-hot per core (zero rows for padding nodes) + 1/count
    oh = np.zeros((NC_, NSH, N_GRAPHS), np.float32)
    for c in range(NC_):
        oh[c][rank[c], b_all[c * NPC:(c + 1) * NPC]] = 1.0
    oh = oh.astype(BF16)
    cnt = np.bincount(b_all, minlength=N_GRAPHS).astype(np.float32)
    inv_cnt = (1.0 / np.maximum(cnt, 1.0)).reshape(N_GRAPHS, 1)

    return dict(D=D, ss=ss, S=S, idx16=idx16, xT=xT, oh=oh, inv_cnt=inv_cnt)


# ----------------------------------------------------------------------------
# Device program
# ----------------------------------------------------------------------------

def _build(D, S, reps=1, skip_ag=False, skip_gather=False, skip_mac=False):
    import concourse.bass as bass
    import concourse.mybir as mybir
    import concourse.tile as tile
    from concourse import bacc
    from concourse.bass import BassGpSimd, _add_dep_helper
    from concourse.masks import make_identity

    f32 = mybir.dt.float32
    bf16 = mybir.dt.bfloat16
    i16 = mybir.dt.int16
    Alu = mybir.AluOpType
    Act = mybir.ActivationFunctionType
    ss = np.concatenate([[0], np.cumsum(D)]).astype(int)

    nc = bacc.Bacc("TRN2", target_bir_lowering=False, debug=False,
                   num_devices=NC_)

    # ---- I/O ----
    xT = nc.dram_tensor("xT", [P, 3, NSH], bf16, kind="ExternalInput")
    idx = nc.dram_tensor("idx", [P, S * 8], i16, kind="ExternalInput")
    Ws, atts, brs = [], [], []
    for li, (Fin, H, Fout, ELEM) in enumerate(LAYERS):
        Ws.append(nc.dram_tensor(f"W{li}", [Fin, Fout], f32,
                                 kind="ExternalInput"))
        atts.append(nc.dram_tensor(f"att{li}", [Fout, 2 * H], f32,
                                   kind="ExternalInput"))
        brs.append(nc.dram_tensor(f"b{li}", [P, Fout], f32,
                                  kind="ExternalInput"))
    oh = nc.dram_tensor("oh", [NSH, N_GRAPHS], bf16, kind="ExternalInput")
    fcW = nc.dram_tensor("fcW", [P, 3], f32, kind="ExternalInput")
    fcb = nc.dram_tensor("fcb", [N_GRAPHS, 3], f32, kind="ExternalInput")
    invc = nc.dram_tensor("invc", [N_GRAPHS, 1], f32, kind="ExternalInput")
    out = nc.dram_tensor("out", [N_GRAPHS, 3], f32, kind="ExternalOutput")

    rg = [list(range(NC_))]

    with tile.TileContext(nc) as tc:
        with tc.tile_pool(name="const", bufs=1) as cpool, \
             tc.tile_pool(name="dram", bufs=1, space="DRAM") as dram, \
             tc.tile_pool(name="io", bufs=3) as io, \
             tc.tile_pool(name="gth", bufs=2) as gth, \
             tc.tile_pool(name="cmp", bufs=2) as cmp, \
             tc.tile_pool(name="ps", bufs=2, space="PSUM") as ps, \
             tc.tile_pool(name="pacc", bufs=1, space="PSUM") as pacc:

            # ---- DRAM internals ----
            shard = [dram.tile([NSH, ELEM], bf16, name=f"shard{li}")
                     for li, (_, _, _, ELEM) in enumerate(LAYERS)]
            pool_in = dram.tile([N_GRAPHS, P], f32, name="pool_in")

            # ---- constants to SBUF ----
            idx_sb = cpool.tile([P, S * 8], i16, name="idx_sb")
            nc.sync.dma_start(idx_sb[:], idx[:])

            b_sb, ad_all, dummy = [], [], []
            for li, (Fin, H, Fout, ELEM) in enumerate(LAYERS):
                t_ = cpool.tile([P, Fout], f32, name=f"b_sb{li}")
                nc.sync.dma_start(t_[:], brs[li][:])
                b_sb.append(t_)
                ad_all.append(cpool.tile([P, NT * H], f32, name=f"ad{li}"))
                dm = cpool.tile([1, ELEM], bf16, name=f"dummy{li}")
                nc.vector.memset(dm[:], 0.0)
                nc.vector.memset(dm[:, Fout:Fout + H], NEG)
                dummy.append(dm)

            ident_bf = cpool.tile([P, P], bf16, name="ident_bf")
            make_identity(nc, ident_bf[:])
            ident_f = cpool.tile([P, P], f32, name="ident_f")
            make_identity(nc, ident_f[:])
            fcW_sb = cpool.tile([P, 3], f32, name="fcW_sb")
            nc.sync.dma_start(fcW_sb[:], fcW[:])
            fcb_sb = cpool.tile([N_GRAPHS, 3], f32, name="fcb_sb")
            nc.sync.dma_start(fcb_sb[:], fcb[:])
            invc_sb = cpool.tile([N_GRAPHS, 1], f32, name="invc_sb")
            nc.sync.dma_start(invc_sb[:], invc[:])

            # persistent PSUM accumulator for graph pooling
            pool_ps = pacc.tile([N_GRAPHS, P], f32, name="pool_ps")

            # ---- W_ext = [W | W@A_s | W@A_d] (bf16, per layer) ----
            def build_wext(li, rep):
                Fin, H, Fout, ELEM = LAYERS[li]
                KC, FoC = Fin // P, Fout // P
                wext = cpool.tile([P, KC, Fout + 2 * H], bf16,
                                  tag=f"wext{li}", name=f"wext{li}_r{rep}")
                nc.gpsimd.dma_start(
                    wext[:, :, :Fout],
                    Ws[li].ap().rearrange("(k p) f -> p k f", p=P))
                att_sb = cpool.tile([P, FoC, 2 * H], bf16,
                                    tag=f"attsb{li}", name=f"attsb{li}_r{rep}")
                nc.gpsimd.dma_start(
                    att_sb[:],
                    atts[li].ap().rearrange("(c p) h -> p c h", p=P))
                for fic in range(KC):
                    wa_ps = ps.tile([P, 2 * H], f32, tag="hpa")
                    for foc in range(FoC):
                        tp = ps.tile([P, P], bf16, tag="tp")
                        nc.tensor.transpose(
                            tp[:], wext[:, fic, foc * P:(foc + 1) * P],
                            ident_bf[:])
                        wt = cmp.tile([P, P], bf16, tag="wt")
                        nc.vector.tensor_copy(wt[:], tp[:])
                        nc.tensor.matmul(
                            wa_ps[:], lhsT=wt[:], rhs=att_sb[:, foc, :],
                            start=(foc == 0), stop=(foc == FoC - 1))
                    nc.vector.tensor_copy(wext[:, fic, Fout:], wa_ps[:])
                return wext

            def do_ag_chunk(li, j, tbl, ag_insts, eng):
                # AG chunk j: shard rows [j*GR,(j+1)*GR) of every core ->
                # tbl rows [j*8GR,(j+1)*8GR).  tbl is a RAW Shared DRAM
                # tensor (outside Tile tracking, which rejects multiple
                # collective writers); consumers get explicit dep edges.
                # Issued from a lightly-used engine so its sem-wait never
                # stalls the gather/compute streams.
                if skip_ag:
                    inst = nc.sync.dma_start(
                        tbl[li][j * NC_ * GR:j * NC_ * GR + GR, :],
                        shard[li][j * GR:(j + 1) * GR])
                else:
                    inst = BassGpSimd.collective_compute(
                        eng, "AllGather", Alu.bypass, replica_groups=rg,
                        ins=[shard[li][j * GR:(j + 1) * GR]],
                        outs=[tbl[li][j * NC_ * GR:(j + 1) * NC_ * GR, :]])
                ag_insts[li].append(inst)

            def maybe_ag(li, t, tbl, ag_insts, eng):
                # after tile t of the producing stage, AG the completed chunk
                if (t + 1) % G:
                    return
                j = t // G
                if j == NCH - 1:
                    nc.sync.dma_start(
                        shard[li][DUMMY:DUMMY + 1, :], dummy[li][:])
                do_ag_chunk(li, j, tbl, ag_insts, eng)

            # ---- shared tail: h/a in PSUM -> table row + local a_d ----
            def layer_tail(li, t, hp, hpa):
                Fin, H, Fout, ELEM = LAYERS[li]
                nc.scalar.copy(
                    ad_all[li][:, t * H:(t + 1) * H], hpa[:, H:2 * H])
                row = io.tile([P, ELEM], bf16, tag="row")
                nc.vector.tensor_tensor(
                    out=row[:, :Fout], in0=hp[:, :Fout], in1=b_sb[li][:],
                    op=Alu.add)
                nc.scalar.copy(row[:, Fout:Fout + H], hpa[:, :H])
                if ELEM > Fout + H:
                    nc.vector.memset(row[:, Fout + H:], 0.0)
                nc.sync.dma_start(shard[li][t * P:(t + 1) * P, :], row[:])

            for rep in range(reps):
                tbl = [nc.dram_tensor(f"tbl{li}_r{rep}", [V, ELEM], bf16,
                                      kind="Internal", addr_space="Shared")
                       for li, (_, _, _, ELEM) in enumerate(LAYERS)]
                ag_insts = [[] for _ in LAYERS]
                pool_out = dram.tile([N_GRAPHS, P], f32, addr_space="Shared",
                                     name=f"pool_out_r{rep}")
                wexts = [build_wext(li, rep) for li in range(len(LAYERS))]

                # ---- stage 0: L1 matmul over own shard ----
                for t in range(NT):
                    xt = io.tile([P, 3, P], bf16, tag="xt")
                    nc.sync.dma_start(xt[:], xT[:, :, t * P:(t + 1) * P])
                    Fout0, H0 = LAYERS[0][2], LAYERS[0][1]
                    hp = ps.tile([P, Fout0], f32, tag="hp")
                    hpa = ps.tile([P, 2 * H0], f32, tag="hpa")
                    for kc in range(3):
                        nc.tensor.matmul(
                            hp[:], lhsT=xt[:, kc, :],
                            rhs=wexts[0][:, kc, :Fout0],
                            start=(kc == 0), stop=(kc == 2))
                        nc.tensor.matmul(
                            hpa[:], lhsT=xt[:, kc, :],
                            rhs=wexts[0][:, kc, Fout0:],
                            start=(kc == 0), stop=(kc == 2))
                    layer_tail(0, t, hp, hpa)
                    maybe_ag(0, t, tbl, ag_insts, nc.gpsimd)

                # ---- stages 1..3: aggregate layer li, then matmul li+1 ----
                for li, (Fin, H, Fout, ELEM) in enumerate(LAYERS):
                    last = li == len(LAYERS) - 1
                    for t in range(NT):
                        Dt = D[t]
                        g = gth.tile([P, Dt, ELEM], bf16, tag="g")
                        nc.sync.dma_start(
                            g[:, 0, :], shard[li][t * P:(t + 1) * P, :])
                        if skip_gather:
                            pass
                        else:
                            for gs in range(1, Dt, GCHUNK[li]):
                                gch = min(GCHUNK[li], Dt - gs)
                                gi = nc.gpsimd.dma_gather(
                                    g[:, gs:gs + gch, :], tbl[li][:],
                                    idx_sb[:, int(ss[t] + gs) * 8:
                                           int(ss[t] + gs + gch) * 8],
                                    num_idxs=P * gch, num_idxs_reg=P * gch,
                                    elem_size=ELEM, elem_step=ELEM)
                                if t == 0:
                                    # tbl is raw (untracked) DRAM: order the
                                    # first tile's gathers after every AG
                                    # chunk; later Pool insts follow in order
                                    for cc in ag_insts[li]:
                                        _add_dep_helper(
                                            gi.ins, cc.ins, sync=True,
                                            reason="tbl chunks ready")
                        # issue the AG for the chunk completed one tile ago —
                        # AFTER tile t's gathers are queued on Pool, so the
                        # AG's sem-wait never blocks gather prefetch
                        if (not last) and t > 0 and t % G == 0:
                            do_ag_chunk(li + 1, t // G - 1, tbl, ag_insts,
                                        nc.gpsimd)

                        # fused segment softmax; e layout [P, Dt, H]
                        e0 = cmp.tile([P, Dt, H], f32, tag="e0")
                        ad_b = ad_all[li][:, t * H:(t + 1) * H] \
                            .unsqueeze(1).broadcast_to([P, Dt, H])
                        nc.vector.tensor_tensor(
                            out=e0[:], in0=g[:, :, Fout:Fout + H],
                            in1=ad_b, op=Alu.add)
                        e1 = cmp.tile([P, Dt, H], f32, tag="e1")
                        nc.vector.scalar_tensor_tensor(
                            out=e1[:], in0=e0[:], scalar=0.2, in1=e0[:],
                            op0=Alu.mult, op1=Alu.max)
                        negm = cmp.tile([P, H], f32, tag="negm")
                        nc.vector.tensor_reduce(
                            out=negm[:],
                            in_=e1[:].rearrange("p s h -> p h s"),
                            axis=mybir.AxisListType.X, op=Alu.max,
                            negate=True)
                        negm_b = negm[:].unsqueeze(1).broadcast_to(
                            [P, Dt, H])
                        nc.vector.tensor_tensor(
                            out=e0[:], in0=e1[:], in1=negm_b, op=Alu.add)
                        w_ = cmp.tile([P, Dt, H], f32, tag="w_")
                        nc.scalar.activation(w_[:], e0[:], Act.Exp)
                        den = cmp.tile([P, H], f32, tag="den")
                        nc.vector.tensor_reduce(
                            out=den[:],
                            in_=w_[:].rearrange("p s h -> p h s"),
                            axis=mybir.AxisListType.X, op=Alu.add)
                        invden = cmp.tile([P, H], f32, tag="invden")
                        nc.vector.reciprocal(invden[:], den[:])
                        # alpha duplicated pairwise so the MAC multiply's
                        # operands are all innermost-packed (DVE 2x mode)
                        alpha = cmp.tile([P, Dt, H, 2], bf16, tag="alpha")
                        w_b = w_[:].unsqueeze(3).broadcast_to([P, Dt, H, 2])
                        invden_b = invden[:].unsqueeze(1).broadcast_to(
                            [P, Dt, H]).unsqueeze(3).broadcast_to(
                            [P, Dt, H, 2])
                        nc.vector.tensor_tensor(
                            out=alpha[:], in0=w_b, in1=invden_b,
                            op=Alu.mult)

                        # acc[p,f] = sum_s alpha[p,s,head(f)] * h[p,s,f]
                        acc = cmp.tile([P, Fout], f32, tag="acc")
                        acc2 = cmp.tile([P, Fout], f32, tag="acc2")
                        if skip_mac:
                            nc.vector.tensor_copy(acc[:], g[:, 0, :Fout])
                        for ci, cs in enumerate(
                                [] if skip_mac else
                                list(range(0, Dt, SCHUNK))):
                            ch = min(SCHUNK, Dt - cs)
                            sc = cmp.tile([P, SCHUNK * Fout], bf16, tag="sc")
                            g_v = g[:, cs:cs + ch, :Fout].rearrange(
                                "p c (h j k) -> p c h j k", h=H, j=P // 2)
                            w_v = alpha[:, cs:cs + ch, :, :].unsqueeze(
                                3).broadcast_to([P, ch, H, P // 2, 2])
                            s_v = sc[:, :ch * Fout].rearrange(
                                "p (c h j k) -> p c h j k",
                                c=ch, h=H, j=P // 2)
                            nc.vector.tensor_tensor(
                                out=s_v, in0=g_v, in1=w_v, op=Alu.mult)
                            # slot-sum as a tree of packed bf16 adds
                            # (tensor_reduce has no fast DVE mode)
                            cur = ch
                            while cur > 1:
                                hh = cur // 2
                                lo = cur - hh
                                nc.vector.tensor_tensor(
                                    out=sc[:, :hh * Fout],
                                    in0=sc[:, :hh * Fout],
                                    in1=sc[:, lo * Fout:(lo + hh) * Fout],
                                    op=Alu.add)
                                cur = lo
                            if ci == 0:
                                nc.scalar.copy(acc[:], sc[:, :Fout])
                            else:
                                nc.vector.tensor_tensor(
                                    out=acc[:], in0=acc[:], in1=sc[:, :Fout],
                                    op=Alu.add)

                        # x_next = elu(acc)   (bias already in table rows)
                        txm = cmp.tile([P, Fout], f32, tag="txm")
                        nc.vector.tensor_scalar_min(txm[:], acc[:], 0.0)
                        texp = cmp.tile([P, Fout], f32, tag="texp")
                        nc.scalar.activation(texp[:], txm[:], Act.Exp)
                        txr = cmp.tile([P, Fout], f32, tag="txr")
                        nc.scalar.activation(txr[:], acc[:], Act.Relu)
                        xn = cmp.tile([P, Fout], bf16, tag="xn")
                        nc.vector.scalar_tensor_tensor(
                            out=xn[:], in0=texp[:], scalar=-1.0, in1=txr[:],
                            op0=Alu.add, op1=Alu.add)

                        if not last:
                            Fin2, H2, Fout2, ELEM2 = LAYERS[li + 1]
                            KC2 = Fin2 // P
                            xtT = io.tile([P, KC2, P], bf16, tag="xtT")
                            for kc in range(KC2):
                                tp = ps.tile([P, P], bf16, tag="tp")
                                nc.tensor.transpose(
                                    tp[:], xn[:, kc * P:(kc + 1) * P],
                                    ident_bf[:])
                                nc.scalar.copy(xtT[:, kc, :], tp[:])
                            hp2 = ps.tile([P, Fout2], f32, tag="hp")
                            hpa2 = ps.tile([P, 2 * H2], f32, tag="hpa")
                            for kc in range(KC2):
                                nc.tensor.matmul(
                                    hp2[:], lhsT=xtT[:, kc, :],
                                    rhs=wexts[li + 1][:, kc, :Fout2],
                                    start=(kc == 0), stop=(kc == KC2 - 1))
                                nc.tensor.matmul(
                                    hpa2[:], lhsT=xtT[:, kc, :],
                                    rhs=wexts[li + 1][:, kc, Fout2:],
                                    start=(kc == 0), stop=(kc == KC2 - 1))
                            layer_tail(li + 1, t, hp2, hpa2)
                        else:
                            oh_t = io.tile([P, N_GRAPHS], bf16, tag="oh_t")
                            nc.sync.dma_start(
                                oh_t[:], oh[t * P:(t + 1) * P, :])
                            nc.tensor.matmul(
                                pool_ps[:], lhsT=oh_t[:], rhs=xn[:],
                                start=(t == 0), stop=(t == NT - 1))

                    if not last:
                        # penultimate chunk (issued during the loop covers up
                        # to chunk NCH-2); flush it and the final chunk
                        nc.sync.dma_start(
                            shard[li + 1][DUMMY:DUMMY + 1, :],
                            dummy[li + 1][:])
                        do_ag_chunk(li + 1, NCH - 1, tbl, ag_insts, nc.gpsimd)



                # ---- finale: mean pool + FC ----
                pool_sb = cmp.tile([N_GRAPHS, P], f32, tag="pool_sb")
                nc.vector.tensor_copy(pool_sb[:], pool_ps[:])
                nc.sync.dma_start(pool_in[:], pool_sb[:])
                if skip_ag:
                    nc.sync.dma_start(pool_out[:], pool_in[:])
                else:
                    nc.gpsimd.collective_compute(
                        "AllReduce", Alu.add, replica_groups=rg,
                        ins=[pool_in[:]], outs=[pool_out[:]])
                pr = cmp.tile([N_GRAPHS, P], f32, tag="pr")
                nc.sync.dma_start(pr[:], pool_out[:])
                pm = cmp.tile([N_GRAPHS, P], f32, tag="pm")
                nc.vector.tensor_scalar_mul(pm[:], pr[:], invc_sb[:, 0:1])
                tp2 = ps.tile([P, N_GRAPHS], f32, tag="tp")
                nc.tensor.transpose(
                    tp2[:], pm[:], ident_f[:N_GRAPHS, :N_GRAPHS])
                pmT = cmp.tile([P, N_GRAPHS], f32, tag="pmT")
                nc.vector.tensor_copy(pmT[:], tp2[:])
                fc_ps = ps.tile([N_GRAPHS, 3], f32, tag="tp")
                nc.tensor.matmul(fc_ps[:], lhsT=pmT[:], rhs=fcW_sb[:],
                                 start=True, stop=True)
                res = cmp.tile([N_GRAPHS, 3], f32, tag="res")
                nc.vector.tensor_tensor(
                    out=res[:], in0=fc_ps[:], in1=fcb_sb[:], op=Alu.add)
                nc.sync.dma_start(out[:], res[:])

    nc.compile()
    return nc


# ----------------------------------------------------------------------------
# Entry point
# ----------------------------------------------------------------------------

def _make_in_maps(prep, params, fcW, fcb):
    common = {}
    for li, (W, as_, ad_, b_) in enumerate(params):
        Fin, H, Fout, ELEM = LAYERS[li]
        common[f"W{li}"] = np.ascontiguousarray(np.asarray(W, np.float32))
        # block-diagonal att matrices: att[k*128+j, k] = a_s[k, j] etc.
        att = np.zeros((Fout, 2 * H), np.float32)
        as_ = np.asarray(as_, np.float32)
        ad_ = np.asarray(ad_, np.float32)
        for k in range(H):
            att[k * P:(k + 1) * P, k] = as_[k]
            att[k * P:(k + 1) * P, H + k] = ad_[k]
        common[f"att{li}"] = att
        common[f"b{li}"] = np.tile(
            np.asarray(b_, np.float32).reshape(1, Fout), (P, 1))
    common["fcW"] = np.ascontiguousarray(np.asarray(fcW, np.float32))
    common["fcb"] = np.tile(
        np.asarray(fcb, np.float32).reshape(1, 3), (N_GRAPHS, 1))
    common["invc"] = prep["inv_cnt"].astype(np.float32)

    in_maps = []
    for c in range(NC_):
        m = dict(common)
        m["xT"] = np.ascontiguousarray(prep["xT"][c])
        m["idx"] = np.ascontiguousarray(prep["idx16"][c])
        m["oh"] = np.ascontiguousarray(prep["oh"][c])
        in_maps.append(m)
    return in_maps


def kernel(x, edge_index, batch, W1, as1, ad1, b1, W2, as2, ad2, b2,
           W3, as3, ad3, b3, fcW, fcb, _trace=False):
    from concourse.bass_utils import run_bass_kernel_spmd

    prep = _prep(x, edge_index, batch)
    D, S = prep["D"], prep["S"]

    key = (tuple(D), S)
    if key not in _CACHE:
        _CACHE[key] = _build(D, S)
    nc = _CACHE[key]

    params = [(W1, as1, ad1, b1), (W2, as2, ad2, b2), (W3, as3, ad3, b3)]
    in_maps = _make_in_maps(prep, params, fcW, fcb)

    res = run_bass_kernel_spmd(nc, in_maps, core_ids=list(range(NC_)),
                               trace=_trace)
    out = res.results[0]["out"].astype(np.float32)
    if _trace:
        kernel._last_results = res
    return out



# revision 27
# speedup vs baseline: 2.8781x; 1.1283x over previous
"""3-layer GAT classifier on 8 TRN2 NeuronCores (Bass/Tile).

Strategy (per spec sharding hint): destination nodes are partitioned across
the 8 cores (2500 real nodes each, padded to 2560 = 20 tiles of 128).
Within a core, destinations are sorted by in-degree so each 128-dst tile has
near-uniform degree (padded-CSR with per-tile max degree D_t, globally
uniform across cores so the SPMD program is identical everywhere).

Per layer l:
  1. Each core computes h' = x @ [W | W@A_s | W@A_d] for its own shard on
     the PE (bf16; the A-extension columns give per-node attention scores
     a_s, a_d directly; W@A_* is computed on-device once per layer).
     It writes bf16 table rows [h + bias | a_s | pad] to a DRAM shard
     (bias folds into the table because softmax weights sum to 1).
     Row 2559 of every shard is a dummy row (h=0, a_s=-1e30) used for CSR
     padding slots.
  2. AllGather -> full gather table [20480, elem] on every core.
  3. Per dst tile: chunked dma_gather pulls all incident-edge rows
     (slot-major padded CSR, 128 edges per slot; <=1024 idx / ~1MB per op,
     a HW limit).  Segment softmax runs as ~8 fused broadcast-AP ops per
     tile (per-head structure handled by strided/step-0 APs; 1/den folds
     into alpha).  The weighted sum is a broadcast multiply (bf16) +
     strided reduce (fp32) per 16-slot chunk.  Output feeds the next
     layer's matmul via PE transpose, all on-chip.
Final: mean-pool by graph via one-hot matmul (PSUM-accumulated across
tiles), AllReduce, tiny FC -> [64, 3].

Host-side work is limited to index manipulation (edge grouping, padded CSR
construction, one-hot graph membership, 1/count) plus dtype/layout staging
of the inputs.
"""

import sys

sys.path.insert(0, "/opt/trn_rl_repo")

import numpy as np
import ml_dtypes

N_NODES = 20000
N_EDGES = 320000
N_GRAPHS = 64
NC_ = 8          # cores
P = 128          # partitions
NPC = 2500       # real nodes per core
NSH = 2560       # padded shard rows per core
NT = NSH // P    # 20 dst tiles per core
V = NC_ * NSH    # 20480 gather-table rows
G = 4            # tiles per AllGather chunk (pipelined with compute)
GR = G * P       # shard rows per AllGather chunk
NCH = NT // G    # chunks per layer
DUMMY = NSH - 1  # shard row used for padding slots (core 0's)
# gather-table position of core 0's dummy shard row in the chunked layout
DUMMY_POS = ((NSH - 1) // GR) * NC_ * GR + (NSH - 1) % GR
NEG = -1.0e30

# (Fin, H, Fout, ELEM) per layer; ELEM = bf16 elements per table row,
# padded so ELEM*2 bytes is a multiple of 256.
LAYERS = [
    (384, 4, 512, 640),
    (512, 2, 256, 384),
    (256, 1, 128, 256),
]

SCHUNK = 12  # slots per weighted-sum chunk (bounds the `scaled` tile)
# slots (128 idxs each) per dma_gather op; HW crashes above ~1024 idxs or
# ~1.1MB per gather op (empirical, see probe_hw.py)
GCHUNK = [7, 8, 8]

BF16 = ml_dtypes.bfloat16

_CACHE = {}


# ----------------------------------------------------------------------------
# Host-side preprocessing (index manipulation only)
# ----------------------------------------------------------------------------

def _prep(x, edge_index, batch):
    x = np.asarray(x, dtype=np.float32)
    ei = np.asarray(edge_index)
    b_all = np.asarray(batch).astype(np.int64)

    loop = np.arange(N_NODES, dtype=np.int64)
    src = np.concatenate([ei[0].astype(np.int64), loop])
    dst = np.concatenate([ei[1].astype(np.int64), loop])

    cd = dst // NPC
    ld = dst % NPC

    deg = np.zeros((NC_, NPC), np.int64)
    np.add.at(deg, (cd, ld), 1)
    order = np.argsort(-deg, axis=1, kind="stable")      # [NC_, NPC]
    rank = np.empty_like(order)
    for c in range(NC_):
        rank[c, order[c]] = np.arange(NPC)

    degsort = np.take_along_axis(deg, order, axis=1)
    degsort = np.concatenate(
        [degsort, np.zeros((NC_, NSH - NPC), np.int64)], axis=1)
    D = [int(max(1, degsort[:, t * P:(t + 1) * P].max())) for t in range(NT)]
    ss = np.concatenate([[0], np.cumsum(D)]).astype(np.int64)
    S = int(ss[-1])

    nodes = np.arange(N_NODES)
    c_n = nodes // NPC
    r_n = rank[c_n, nodes % NPC]
    # chunked-AG table layout: [chunk][core][row-in-chunk]
    pos = (r_n // GR) * (NC_ * GR) + c_n * GR + (r_n % GR)  # [N]

    # place each edge at (core, slot, partition); self-loop edge first
    # (slot 0 is loaded from the local shard by a plain DMA, not gathered)
    key = cd * NSH + rank[cd, ld]
    eo = np.lexsort(((src != dst).astype(np.int8), key))
    ks = key[eo]
    first = np.searchsorted(ks, ks, side="left")
    sidx = np.arange(len(ks)) - first                    # within-dst slot
    ce = ks // NSH
    re = ks % NSH
    te = re // P
    pe = re % P
    slot = ss[te] + sidx
    full = np.full((NC_, S, P), DUMMY_POS, np.int16)
    full[ce, slot, pe] = pos[src[eo]].astype(np.int16)

    # dma_gather index layout: idx i at [i % 16, i // 16], replicated x8
    idxw = full.reshape(NC_, S * 8, 16).transpose(0, 2, 1)   # [NC_,16,S*8]
    idx16 = np.ascontiguousarray(np.tile(idxw, (1, 8, 1)))   # [NC_,128,S*8]

    # x in pos order, transposed for the L1 matmul: [c, 128f, 3kc, NSH]
    xp = np.zeros((NC_, NSH, 384), np.float32)
    for c in range(NC_):
        xp[c, rank[c], :] = x[c * NPC:(c + 1) * NPC]
    xT = np.ascontiguousarray(
        xp.transpose(0, 2, 1).reshape(NC_, 3, P, NSH).transpose(0, 2, 1, 3)
    ).astype(BF16)

    # graph one-hot per core (zero rows for padding nodes) + 1/count
    oh = np.zeros((NC_, NSH, N_GRAPHS), np.float32)
    for c in range(NC_):
        oh[c][rank[c], b_all[c * NPC:(c + 1) * NPC]] = 1.0
    oh = oh.astype(BF16)
    cnt = np.bincount(b_all, minlength=N_GRAPHS).astype(np.float32)
    inv_cnt = (1.0 / np.maximum(cnt, 1.0)).reshape(N_GRAPHS, 1)

    return dict(D=D, ss=ss, S=S, idx16=idx16, xT=xT, oh=oh, inv_cnt=inv_cnt)


# ----------------------------------------------------------------------------
# Device program
# ----------------------------------------------------------------------------

def _build(D, S, reps=1, skip_ag=False, skip_gather=False, skip_mac=False,
           gbufs=3, nq=2, fast_softmax=False):
    import concourse.bass as bass
    import concourse.mybir as mybir
    import concourse.tile as tile
    from concourse import bacc
    from concourse.bass import BassGpSimd, _add_dep_helper
    from concourse.masks import make_identity

    f32 = mybir.dt.float32
    bf16 = mybir.dt.bfloat16
    i16 = mybir.dt.int16
    Alu = mybir.AluOpType
    Act = mybir.ActivationFunctionType
    ss = np.concatenate([[0], np.cumsum(D)]).astype(int)

    nc = bacc.Bacc("TRN2", target_bir_lowering=False, debug=False,
                   num_devices=NC_, num_swdge_queues=nq)

    # ---- I/O ----
    xT = nc.dram_tensor("xT", [P, 3, NSH], bf16, kind="ExternalInput")
    idx = nc.dram_tensor("idx", [P, S * 8], i16, kind="ExternalInput")
    Ws, atts, brs = [], [], []
    for li, (Fin, H, Fout, ELEM) in enumerate(LAYERS):
        Ws.append(nc.dram_tensor(f"W{li}", [Fin, Fout], f32,
                                 kind="ExternalInput"))
        atts.append(nc.dram_tensor(f"att{li}", [Fout, 2 * H], f32,
                                   kind="ExternalInput"))
        brs.append(nc.dram_tensor(f"b{li}", [P, Fout], f32,
                                  kind="ExternalInput"))
    oh = nc.dram_tensor("oh", [NSH, N_GRAPHS], bf16, kind="ExternalInput")
    fcW = nc.dram_tensor("fcW", [P, 3], f32, kind="ExternalInput")
    fcb = nc.dram_tensor("fcb", [N_GRAPHS, 3], f32, kind="ExternalInput")
    invc = nc.dram_tensor("invc", [N_GRAPHS, 1], f32, kind="ExternalInput")
    out = nc.dram_tensor("out", [N_GRAPHS, 3], f32, kind="ExternalOutput")

    rg = [list(range(NC_))]

    with tile.TileContext(nc) as tc:
        with tc.tile_pool(name="const", bufs=1) as cpool, \
             tc.tile_pool(name="dram", bufs=1, space="DRAM") as dram, \
             tc.tile_pool(name="io", bufs=3) as io, \
             tc.tile_pool(name="gth", bufs=gbufs) as gth, \
             tc.tile_pool(name="cmp", bufs=2) as cmp, \
             tc.tile_pool(name="ps", bufs=2, space="PSUM") as ps, \
             tc.tile_pool(name="pacc", bufs=1, space="PSUM") as pacc:

            # ---- DRAM internals ----
            shard = [dram.tile([NSH, ELEM], bf16, name=f"shard{li}")
                     for li, (_, _, _, ELEM) in enumerate(LAYERS)]
            pool_in = dram.tile([N_GRAPHS, P], f32, name="pool_in")

            # ---- constants to SBUF ----
            idx_sb = cpool.tile([P, S * 8], i16, name="idx_sb")
            nc.sync.dma_start(idx_sb[:], idx[:])

            b_sb, ad_all, dummy = [], [], []
            for li, (Fin, H, Fout, ELEM) in enumerate(LAYERS):
                t_ = cpool.tile([P, Fout], f32, name=f"b_sb{li}")
                nc.sync.dma_start(t_[:], brs[li][:])
                b_sb.append(t_)
                ad_all.append(cpool.tile([P, NT * H], f32, name=f"ad{li}"))
                dm = cpool.tile([1, ELEM], bf16, name=f"dummy{li}")
                nc.vector.memset(dm[:], 0.0)
                nc.vector.memset(dm[:, Fout:Fout + H], NEG)
                dummy.append(dm)

            ident_bf = cpool.tile([P, P], bf16, name="ident_bf")
            make_identity(nc, ident_bf[:])
            ident_f = cpool.tile([P, P], f32, name="ident_f")
            make_identity(nc, ident_f[:])
            fcW_sb = cpool.tile([P, 3], f32, name="fcW_sb")
            nc.sync.dma_start(fcW_sb[:], fcW[:])
            fcb_sb = cpool.tile([N_GRAPHS, 3], f32, name="fcb_sb")
            nc.sync.dma_start(fcb_sb[:], fcb[:])
            invc_sb = cpool.tile([N_GRAPHS, 1], f32, name="invc_sb")
            nc.sync.dma_start(invc_sb[:], invc[:])

            # persistent PSUM accumulator for graph pooling
            pool_ps = pacc.tile([N_GRAPHS, P], f32, name="pool_ps")

            # ---- W_ext = [W | W@A_s | W@A_d] (bf16, per layer) ----
            def build_wext(li, rep):
                Fin, H, Fout, ELEM = LAYERS[li]
                KC, FoC = Fin // P, Fout // P
                wext = cpool.tile([P, KC, Fout + 2 * H], bf16,
                                  tag=f"wext{li}", name=f"wext{li}_r{rep}")
                nc.gpsimd.dma_start(
                    wext[:, :, :Fout],
                    Ws[li].ap().rearrange("(k p) f -> p k f", p=P))
                att_sb = cpool.tile([P, FoC, 2 * H], bf16,
                                    tag=f"attsb{li}", name=f"attsb{li}_r{rep}")
                nc.gpsimd.dma_start(
                    att_sb[:],
                    atts[li].ap().rearrange("(c p) h -> p c h", p=P))
                for fic in range(KC):
                    wa_ps = ps.tile([P, 2 * H], f32, tag="hpa")
                    for foc in range(FoC):
                        tp = ps.tile([P, P], bf16, tag="tp")
                        nc.tensor.transpose(
                            tp[:], wext[:, fic, foc * P:(foc + 1) * P],
                            ident_bf[:])
                        wt = cmp.tile([P, P], bf16, tag="wt")
                        nc.vector.tensor_copy(wt[:], tp[:])
                        nc.tensor.matmul(
                            wa_ps[:], lhsT=wt[:], rhs=att_sb[:, foc, :],
                            start=(foc == 0), stop=(foc == FoC - 1))
                    nc.vector.tensor_copy(wext[:, fic, Fout:], wa_ps[:])
                return wext

            def do_ag_chunk(li, j, tbl, ag_insts, eng):
                # AG chunk j: shard rows [j*GR,(j+1)*GR) of every core ->
                # tbl rows [j*8GR,(j+1)*8GR).  tbl is a RAW Shared DRAM
                # tensor (outside Tile tracking, which rejects multiple
                # collective writers); consumers get explicit dep edges.
                # Issued from a lightly-used engine so its sem-wait never
                # stalls the gather/compute streams.
                if skip_ag:
                    inst = nc.sync.dma_start(
                        tbl[li][j * NC_ * GR:j * NC_ * GR + GR, :],
                        shard[li][j * GR:(j + 1) * GR])
                else:
                    inst = BassGpSimd.collective_compute(
                        eng, "AllGather", Alu.bypass, replica_groups=rg,
                        ins=[shard[li][j * GR:(j + 1) * GR]],
                        outs=[tbl[li][j * NC_ * GR:(j + 1) * NC_ * GR, :]])
                ag_insts[li].append(inst)

            def maybe_ag(li, t, tbl, ag_insts, eng):
                # after tile t of the producing stage, AG the completed chunk
                if (t + 1) % G:
                    return
                j = t // G
                if j == NCH - 1:
                    nc.sync.dma_start(
                        shard[li][DUMMY:DUMMY + 1, :], dummy[li][:])
                do_ag_chunk(li, j, tbl, ag_insts, eng)

            # ---- shared tail: h/a in PSUM -> table row + local a_d ----
            def layer_tail(li, t, hp, hpa):
                Fin, H, Fout, ELEM = LAYERS[li]
                nc.scalar.copy(
                    ad_all[li][:, t * H:(t + 1) * H], hpa[:, H:2 * H])
                row = io.tile([P, ELEM], bf16, tag="row")
                nc.vector.tensor_tensor(
                    out=row[:, :Fout], in0=hp[:, :Fout], in1=b_sb[li][:],
                    op=Alu.add)
                nc.scalar.copy(row[:, Fout:Fout + H], hpa[:, :H])
                if ELEM > Fout + H:
                    nc.vector.memset(row[:, Fout + H:], 0.0)
                nc.sync.dma_start(shard[li][t * P:(t + 1) * P, :], row[:])

            gq = [0]  # round-robin gather queue counter
            for rep in range(reps):
                tbl = [nc.dram_tensor(f"tbl{li}_r{rep}", [V, ELEM], bf16,
                                      kind="Internal", addr_space="Shared")
                       for li, (_, _, _, ELEM) in enumerate(LAYERS)]
                ag_insts = [[] for _ in LAYERS]
                pool_out = dram.tile([N_GRAPHS, P], f32, addr_space="Shared",
                                     name=f"pool_out_r{rep}")
                wexts = [build_wext(li, rep) for li in range(len(LAYERS))]

                # ---- stage 0: L1 matmul over own shard ----
                for t in range(NT):
                    xt = io.tile([P, 3, P], bf16, tag="xt")
                    nc.sync.dma_start(xt[:], xT[:, :, t * P:(t + 1) * P])
                    Fout0, H0 = LAYERS[0][2], LAYERS[0][1]
                    hp = ps.tile([P, Fout0], f32, tag="hp")
                    hpa = ps.tile([P, 2 * H0], f32, tag="hpa")
                    for kc in range(3):
                        nc.tensor.matmul(
                            hp[:], lhsT=xt[:, kc, :],
                            rhs=wexts[0][:, kc, :Fout0],
                            start=(kc == 0), stop=(kc == 2))
                        nc.tensor.matmul(
                            hpa[:], lhsT=xt[:, kc, :],
                            rhs=wexts[0][:, kc, Fout0:],
                            start=(kc == 0), stop=(kc == 2))
                    layer_tail(0, t, hp, hpa)
                    maybe_ag(0, t, tbl, ag_insts, nc.gpsimd)

                # ---- stages 1..3: aggregate layer li, then matmul li+1 ----
                for li, (Fin, H, Fout, ELEM) in enumerate(LAYERS):
                    last = li == len(LAYERS) - 1
                    for t in range(NT):
                        Dt = D[t]
                        g = gth.tile([P, Dt, ELEM], bf16, tag="g")
                        nc.sync.dma_start(
                            g[:, 0, :], shard[li][t * P:(t + 1) * P, :])
                        if skip_gather:
                            pass
                        else:
                            for gs in range(1, Dt, GCHUNK[li]):
                                gch = min(GCHUNK[li], Dt - gs)
                                gi = nc.gpsimd.dma_gather(
                                    g[:, gs:gs + gch, :], tbl[li][:],
                                    idx_sb[:, int(ss[t] + gs) * 8:
                                           int(ss[t] + gs + gch) * 8],
                                    num_idxs=P * gch, num_idxs_reg=P * gch,
                                    elem_size=ELEM, elem_step=ELEM,
                                    queue_num=gq[0] % nq)
                                gq[0] += 1
                                if t == 0:
                                    # tbl is raw (untracked) DRAM: order the
                                    # first tile's gathers after every AG
                                    # chunk; later Pool insts follow in order
                                    for cc in ag_insts[li]:
                                        _add_dep_helper(
                                            gi.ins, cc.ins, sync=True,
                                            reason="tbl chunks ready")
                        # issue the AG for the chunk completed one tile ago —
                        # AFTER tile t's gathers are queued on Pool, so the
                        # AG's sem-wait never blocks gather prefetch
                        if (not last) and t > 0 and t % G == 0:
                            do_ag_chunk(li + 1, t // G - 1, tbl, ag_insts,
                                        nc.gpsimd)

                        # fused segment softmax; e layout [P, Dt, H]
                        e0 = cmp.tile([P, Dt, H], f32, tag="e0")
                        ad_b = ad_all[li][:, t * H:(t + 1) * H] \
                            .unsqueeze(1).broadcast_to([P, Dt, H])
                        nc.vector.tensor_tensor(
                            out=e0[:], in0=g[:, :, Fout:Fout + H],
                            in1=ad_b, op=Alu.add)
                        e1 = cmp.tile([P, Dt, H], f32, tag="e1")
                        nc.vector.scalar_tensor_tensor(
                            out=e1[:], in0=e0[:], scalar=0.2, in1=e0[:],
                            op0=Alu.mult, op1=Alu.max)
                        w_ = cmp.tile([P, Dt, H], f32, tag="w_")
                        if fast_softmax:
                            # e is bounded (|a|~few sigma; dummies -1e30 ->
                            # exp 0; slot-0 self row keeps den>0), so skip
                            # the segment-max subtraction entirely
                            nc.scalar.activation(w_[:], e1[:], Act.Exp)
                        else:
                            negm = cmp.tile([P, H], f32, tag="negm")
                            nc.vector.tensor_reduce(
                                out=negm[:],
                                in_=e1[:].rearrange("p s h -> p h s"),
                                axis=mybir.AxisListType.X, op=Alu.max,
                                negate=True)
                            negm_b = negm[:].unsqueeze(1).broadcast_to(
                                [P, Dt, H])
                            nc.vector.tensor_tensor(
                                out=e0[:], in0=e1[:], in1=negm_b, op=Alu.add)
                            nc.scalar.activation(w_[:], e0[:], Act.Exp)
                        den = cmp.tile([P, H], f32, tag="den")
                        nc.vector.tensor_reduce(
                            out=den[:],
                            in_=w_[:].rearrange("p s h -> p h s"),
                            axis=mybir.AxisListType.X, op=Alu.add)
                        invden = cmp.tile([P, H], f32, tag="invden")
                        nc.vector.reciprocal(invden[:], den[:])
                        # alpha duplicated pairwise so the MAC multiply's
                        # operands are all innermost-packed (DVE 2x mode)
                        alpha = cmp.tile([P, Dt, H, 2], bf16, tag="alpha")
                        w_b = w_[:].unsqueeze(3).broadcast_to([P, Dt, H, 2])
                        invden_b = invden[:].unsqueeze(1).broadcast_to(
                            [P, Dt, H]).unsqueeze(3).broadcast_to(
                            [P, Dt, H, 2])
                        nc.vector.tensor_tensor(
                            out=alpha[:], in0=w_b, in1=invden_b,
                            op=Alu.mult)

                        # acc[p,f] = sum_s alpha[p,s,head(f)] * h[p,s,f]
                        acc = cmp.tile([P, Fout], f32, tag="acc")
                        if skip_mac:
                            nc.vector.tensor_copy(acc[:], g[:, 0, :Fout])
                        for ci, cs in enumerate(
                                [] if skip_mac else
                                list(range(0, Dt, SCHUNK))):
                            ch = min(SCHUNK, Dt - cs)
                            sc = cmp.tile([P, SCHUNK * Fout], bf16, tag="sc")
                            g_v = g[:, cs:cs + ch, :Fout].rearrange(
                                "p c (h j k) -> p c h j k", h=H, j=P // 2)
                            w_v = alpha[:, cs:cs + ch, :, :].unsqueeze(
                                3).broadcast_to([P, ch, H, P // 2, 2])
                            s_v = sc[:, :ch * Fout].rearrange(
                                "p (c h j k) -> p c h j k",
                                c=ch, h=H, j=P // 2)
                            nc.vector.tensor_tensor(
                                out=s_v, in0=g_v, in1=w_v, op=Alu.mult)
                            # slot-sum as a tree of packed bf16 adds
                            # (tensor_reduce has no fast DVE mode)
                            cur = ch
                            while cur > 1:
                                hh = cur // 2
                                lo = cur - hh
                                nc.vector.tensor_tensor(
                                    out=sc[:, :hh * Fout],
                                    in0=sc[:, :hh * Fout],
                                    in1=sc[:, lo * Fout:(lo + hh) * Fout],
                                    op=Alu.add)
                                cur = lo
                            if ci == 0:
                                nc.scalar.copy(acc[:], sc[:, :Fout])
                            else:
                                nc.vector.tensor_tensor(
                                    out=acc[:], in0=acc[:], in1=sc[:, :Fout],
                                    op=Alu.add)

                        # x_next = elu(acc)   (bias already in table rows)
                        txm = cmp.tile([P, Fout], f32, tag="txm")
                        nc.vector.tensor_scalar_min(txm[:], acc[:], 0.0)
                        texp = cmp.tile([P, Fout], f32, tag="texp")
                        nc.scalar.activation(texp[:], txm[:], Act.Exp)
                        txr = cmp.tile([P, Fout], f32, tag="txr")
                        nc.scalar.activation(txr[:], acc[:], Act.Relu)
                        xn = cmp.tile([P, Fout], bf16, tag="xn")
                        nc.vector.scalar_tensor_tensor(
                            out=xn[:], in0=texp[:], scalar=-1.0, in1=txr[:],
                            op0=Alu.add, op1=Alu.add)

                        if not last:
                            Fin2, H2, Fout2, ELEM2 = LAYERS[li + 1]
                            KC2 = Fin2 // P
                            xtT = io.tile([P, KC2, P], bf16, tag="xtT")
                            for kc in range(KC2):
                                tp = ps.tile([P, P], bf16, tag="tp")
                                nc.tensor.transpose(
                                    tp[:], xn[:, kc * P:(kc + 1) * P],
                                    ident_bf[:])
                                nc.scalar.copy(xtT[:, kc, :], tp[:])
                            hp2 = ps.tile([P, Fout2], f32, tag="hp")
                            hpa2 = ps.tile([P, 2 * H2], f32, tag="hpa")
                            for kc in range(KC2):
                                nc.tensor.matmul(
                                    hp2[:], lhsT=xtT[:, kc, :],
                                    rhs=wexts[li + 1][:, kc, :Fout2],
                                    start=(kc == 0), stop=(kc == KC2 - 1))
                                nc.tensor.matmul(
                                    hpa2[:], lhsT=xtT[:, kc, :],
                                    rhs=wexts[li + 1][:, kc, Fout2:],
                                    start=(kc == 0), stop=(kc == KC2 - 1))
                            layer_tail(li + 1, t, hp2, hpa2)
                        else:
                            oh_t = io.tile([P, N_GRAPHS], bf16, tag="oh_t")
                            nc.sync.dma_start(
                                oh_t[:], oh[t * P:(t + 1) * P, :])
                            nc.tensor.matmul(
                                pool_ps[:], lhsT=oh_t[:], rhs=xn[:],
                                start=(t == 0), stop=(t == NT - 1))

                    if not last:
                        # penultimate chunk (issued during the loop covers up
                        # to chunk NCH-2); flush it and the final chunk
                        nc.sync.dma_start(
                            shard[li + 1][DUMMY:DUMMY + 1, :],
                            dummy[li + 1][:])
                        do_ag_chunk(li + 1, NCH - 1, tbl, ag_insts, nc.gpsimd)



                # ---- finale: mean pool + FC ----
                pool_sb = cmp.tile([N_GRAPHS, P], f32, tag="pool_sb")
                nc.vector.tensor_copy(pool_sb[:], pool_ps[:])
                nc.sync.dma_start(pool_in[:], pool_sb[:])
                if skip_ag:
                    nc.sync.dma_start(pool_out[:], pool_in[:])
                else:
                    nc.gpsimd.collective_compute(
                        "AllReduce", Alu.add, replica_groups=rg,
                        ins=[pool_in[:]], outs=[pool_out[:]])
                pr = cmp.tile([N_GRAPHS, P], f32, tag="pr")
                nc.sync.dma_start(pr[:], pool_out[:])
                pm = cmp.tile([N_GRAPHS, P], f32, tag="pm")
                nc.vector.tensor_scalar_mul(pm[:], pr[:], invc_sb[:, 0:1])
                tp2 = ps.tile([P, N_GRAPHS], f32, tag="tp")
                nc.tensor.transpose(
                    tp2[:], pm[:], ident_f[:N_GRAPHS, :N_GRAPHS])
                pmT = cmp.tile([P, N_GRAPHS], f32, tag="pmT")
                nc.vector.tensor_copy(pmT[:], tp2[:])
                fc_ps = ps.tile([N_GRAPHS, 3], f32, tag="tp")
                nc.tensor.matmul(fc_ps[:], lhsT=pmT[:], rhs=fcW_sb[:],
                                 start=True, stop=True)
                res = cmp.tile([N_GRAPHS, 3], f32, tag="res")
                nc.vector.tensor_tensor(
                    out=res[:], in0=fc_ps[:], in1=fcb_sb[:], op=Alu.add)
                nc.sync.dma_start(out[:], res[:])

    nc.compile()
    return nc


# ----------------------------------------------------------------------------
# Entry point
# ----------------------------------------------------------------------------

def _make_in_maps(prep, params, fcW, fcb):
    common = {}
    for li, (W, as_, ad_, b_) in enumerate(params):
        Fin, H, Fout, ELEM = LAYERS[li]
        common[f"W{li}"] = np.ascontiguousarray(np.asarray(W, np.float32))
        # block-diagonal att matrices: att[k*128+j, k] = a_s[k, j] etc.
        att = np.zeros((Fout, 2 * H), np.float32)
        as_ = np.asarray(as_, np.float32)
        ad_ = np.asarray(ad_, np.float32)
        for k in range(H):
            att[k * P:(k + 1) * P, k] = as_[k]
            att[k * P:(k + 1) * P, H + k] = ad_[k]
        common[f"att{li}"] = att
        common[f"b{li}"] = np.tile(
            np.asarray(b_, np.float32).reshape(1, Fout), (P, 1))
    common["fcW"] = np.ascontiguousarray(np.asarray(fcW, np.float32))
    common["fcb"] = np.tile(
        np.asarray(fcb, np.float32).reshape(1, 3), (N_GRAPHS, 1))
    common["invc"] = prep["inv_cnt"].astype(np.float32)

    in_maps = []
    for c in range(NC_):
        m = dict(common)
        m["xT"] = np.ascontiguousarray(prep["xT"][c])
        m["idx"] = np.ascontiguousarray(prep["idx16"][c])
        m["oh"] = np.ascontiguousarray(prep["oh"][c])
        in_maps.append(m)
    return in_maps


def kernel(x, edge_index, batch, W1, as1, ad1, b1, W2, as2, ad2, b2,
           W3, as3, ad3, b3, fcW, fcb, _trace=False):
    from concourse.bass_utils import run_bass_kernel_spmd

    prep = _prep(x, edge_index, batch)
    D, S = prep["D"], prep["S"]

    key = (tuple(D), S)
    if key not in _CACHE:
        _CACHE[key] = _build(D, S)
    nc = _CACHE[key]

    params = [(W1, as1, ad1, b1), (W2, as2, ad2, b2), (W3, as3, ad3, b3)]
    in_maps = _make_in_maps(prep, params, fcW, fcb)

    res = run_bass_kernel_spmd(nc, in_maps, core_ids=list(range(NC_)),
                               trace=_trace)
    out = res.results[0]["out"].astype(np.float32)
    if _trace:
        kernel._last_results = res
    return out

